# revision 5
# baseline (speedup 1.0000x reference)
"""Trainium2 Bass kernel for ColorHistogramLoss (anchor + chain scheme).

Reference computation:
  brightness = mean(target, axis=1)           # [B,1,H,W]
  mask = brightness > 0.4
  soft 16-bin Gaussian histograms of pred/target per (b, c), masked,
  normalized; loss = mean |pred_hist - target_hist|.

Kernel strategy (8 NeuronCores, data-parallel over batch B=8):
  Each core processes one image pair (pred[b], target[b]).

  The old baseline evaluated one ScalarE exp per (pixel, bin): 48 ACT ops
  of [128,4096] ~= 182 us — the hard ScalarE floor.  This kernel exploits
  the Gaussian ratio identity between adjacent bins:

      w_{k+1}(x) = w_k(x) * s_k * r(x),
      r = exp((256/15)(x - 1/2)),  s_k = exp((256/15)(1/2 - (c_k+c_{k+1})/2))

  Only 6 ANCHOR bins {0,3,6,9,12,15} get exact exps (ScalarE Square->Exp,
  with the next-hop constant ln s_k folded into the Exp bias so the output
  is pre-scaled W_a = s_a*w_a); the other 10 bins are CHAIN hops:
  one fp16 tensor_tensor (2x mode, W_prev*r) + one fp16 tensor_scalar
  (4x mode) whose fused accum_out is the bin's histogram partial and whose
  scalar slot pre-scales for the next hop.  The host divides each bin's
  partial by its known net scale.

  fp16 error analysis (validated in sim vs the f32 reference: rel err
  4.3e-6 end-to-end): chain weights stay in [e^-17, 2868] — inside fp16
  range; the up-chain + s-scaling conspire so that subnormal underflow
  only happens where the true next-bin weight is also negligible.

  Masking: dark pixels get xm = x + 3 so every anchor sq >= 4 ->
  exp(-128*4) == 0 exactly, and chains propagate the zero.  r is built
  from the unmasked x so it stays finite.

  Engine budget per pair-tile [128, 4096] (2 channel-images):
    DVE : xm stt 4.33 + anchor0 tt 2.19 + 10 chains x (2.19 + 1.13) = 39.7us
    ACT : r 3.6 + 5 x (Square 3.6 + Exp+accum 3.79) + 1 Exp+accum = 44.3us
  x3 pairs + mask (~11.9us DVE): DVE ~131us, ACT ~133us (vs 225/185 before).

Output per core: stats [128, 48] partials; normalize / L1 / mean on host.
"""

from contextlib import ExitStack

import math
import numpy as np

import concourse.bass as bass
import concourse.tile as tile
from concourse import bacc, mybir
from concourse.bass_utils import run_bass_kernel_spmd

N_CORES = 8
C = 3
H = 512
W = 512
HW = H * W          # 262144
P = 128
HP = 64             # partitions per channel in a stacked pair
FP = HW // HP       # 4096
NB = 16
NPAIR = 3           # pair i = (target_i on partitions 0..63, pred_i on 64..127)
F32 = mybir.dt.float32
F16 = mybir.dt.float16

ANCHORS = (0, 3, 6, 9, 12, 15)
CHAINS = {0: (1, 2), 3: (4, 5), 6: (7, 8), 9: (10, 11), 12: (13, 14), 15: ()}
# anchors whose square runs on DVE (ts + tt) instead of ScalarE Square.
# anchor 0 is special: sq = xm*xm needs no ts at all.
E_ANCHORS = (0,)

# s_{k->k+1} = exp((256/15)(1/2 - (c_k + c_{k+1})/2)), c_k = k/15
_S_UP = [math.exp((256.0 / 15.0) * (0.5 - (2 * k + 1) / 30.0)) for k in range(NB - 1)]


def _bin_scale(k):
    """Net scale of the stats partial for bin k (host divides by this)."""
    if k in CHAINS:  # anchor
        return _S_UP[k] if CHAINS[k] else 1.0
    # chain bin: scaled by s_{k->k+1} unless it is the last hop of its chain
    for a, ch in CHAINS.items():
        if k in ch:
            return _S_UP[k] if k != ch[-1] else 1.0
    raise AssertionError(k)


def _kernel_body(ctx, tc, stats_d, pred_d, target_d, repeat=1):
    nc = tc.nc
    pairs = ctx.enter_context(tc.tile_pool(name="pairs", bufs=1))
    maskp = ctx.enter_context(tc.tile_pool(name="maskp", bufs=2))
    scrp = ctx.enter_context(tc.tile_pool(name="scrp", bufs=1))
    xmp = ctx.enter_context(tc.tile_pool(name="xmp", bufs=2))
    rp = ctx.enter_context(tc.tile_pool(name="rp", bufs=2))
    sqp = ctx.enter_context(tc.tile_pool(name="sqp", bufs=2))
    wp = ctx.enter_context(tc.tile_pool(name="wp", bufs=4))
    sp = ctx.enter_context(tc.tile_pool(name="sp", bufs=2))

    # Per-anchor ACT bias constants (ACT bias must be an AP); built once.
    biases = {}
    vals = [-256.0 / 30.0]
    for a in ANCHORS:
        if a not in E_ANCHORS:
            vals.append(-a / 15.0)
        vals.append(math.log(_S_UP[a]) if CHAINS[a] else 0.0)
    vals = sorted(set(vals))
    bias_t = sp.tile([P, len(vals)], F32, tag="bias")
    for i, v in enumerate(vals):
        nc.gpsimd.memset(bias_t[:, i : i + 1], v)
        biases[v] = bias_t[:, i : i + 1]

    pools = (pairs, maskp, scrp, xmp, rp, sqp, wp, sp)
    for _ in range(repeat):
        _emit_pass(tc, pools, biases, stats_d, pred_d, target_d)


def _emit_pass(tc, pools, biases, stats_d, pred_d, target_d):
    nc = tc.nc
    add = mybir.AluOpType.add
    mult = mybir.AluOpType.mult
    subtract = mybir.AluOpType.subtract
    is_le = mybir.AluOpType.is_le
    Exp = mybir.ActivationFunctionType.Exp
    Square = mybir.ActivationFunctionType.Square
    pairs, maskp, scrp, xmp, rp, sqp, wp, sp = pools

    def chan_ap(dram, c):
        return dram[c].rearrange("(q g) -> q g", q=HP)

    # pair i: target_i on partitions 0..63 (mask inputs at base 0), pred_i
    # on 64..127.  Target halves are DMA'd first: the mask gates everything.
    pair_tiles = []
    for i in range(NPAIR):
        t = pairs.tile([P, FP], F32, tag=f"pair{i}")
        nc.sync.dma_start(out=t[:HP, :], in_=chan_ap(target_d, i))
        pair_tiles.append(t)
    for i in range(NPAIR):
        nc.sync.dma_start(out=pair_tiles[i][HP:, :], in_=chan_ap(pred_d, i))

    # mask: moff = (t0+t1+t2 <= 1.2) ? 3.0 : 0.0, replicated to both halves
    scr = scrp.tile([P, FP], F32, tag="scr")
    s01 = scr[:HP, :]
    nc.vector.tensor_tensor(
        out=s01, in0=pair_tiles[0][:HP, :], in1=pair_tiles[1][:HP, :], op=add
    )
    nc.vector.tensor_tensor(out=s01, in0=s01, in1=pair_tiles[2][:HP, :], op=add)
    moff = maskp.tile([P, FP], F32, tag="moff")
    nc.vector.tensor_scalar(
        out=moff[:HP, :], in0=s01, scalar1=1.2, scalar2=3.0, op0=is_le, op1=mult
    )
    nc.vector.tensor_scalar(
        out=moff[HP:, :], in0=moff[:HP, :], scalar1=1.0, scalar2=None, op0=mult
    )

    stats_t = sp.tile([P, NPAIR * NB], F32, tag="stats")

    for pi, x in enumerate(pair_tiles):
        # xm = x + moff (fp16), r = exp((256/15) x - 256/30) (fp16)
        xm = xmp.tile([P, FP], F16, tag="xm")
        nc.vector.scalar_tensor_tensor(
            out=xm[:], in0=x[:], scalar=1.0, in1=moff[:], op0=mult, op1=add
        )
        r = rp.tile([P, FP], F16, tag="r")
        nc.scalar.activation(
            out=r[:], in_=x[:], func=Exp, scale=256.0 / 15.0,
            bias=biases[-256.0 / 30.0],
        )
        for a in ANCHORS:
            col = pi * NB + a
            sq = sqp.tile([P, FP], F16, tag="sq")
            if a in E_ANCHORS:
                if a == 0:
                    nc.vector.tensor_tensor(out=sq[:], in0=xm[:], in1=xm[:], op=mult)
                else:
                    t_ = sqp.tile([P, FP], F16, tag="t")
                    nc.vector.tensor_scalar(
                        out=t_[:], in0=xm[:], scalar1=a / 15.0, scalar2=None,
                        op0=subtract,
                    )
                    nc.vector.tensor_tensor(out=sq[:], in0=t_[:], in1=t_[:], op=mult)
            else:
                nc.scalar.activation(
                    out=sq[:], in_=xm[:], func=Square, scale=1.0,
                    bias=biases[-a / 15.0],
                )
            chain = CHAINS[a]
            bias = math.log(_S_UP[a]) if chain else 0.0
            w_prev = wp.tile([P, FP], F16, tag="w")
            nc.scalar.activation(
                out=w_prev[:], in_=sq[:], func=Exp, scale=-128.0,
                bias=biases[bias],
                accum_out=stats_t[:, col : col + 1],
            )
            for j, b in enumerate(chain):
                wb = wp.tile([P, FP], F16, tag="w")
                nc.vector.tensor_tensor(out=wb[:], in0=w_prev[:], in1=r[:], op=mult)
                last = j == len(chain) - 1
                s_next = 1.0 if last else _S_UP[b]
                wn = wp.tile([P, FP], F16, tag="w")
                nc.vector.tensor_scalar(
                    out=wn[:], in0=wb[:], scalar1=s_next, scalar2=None, op0=mult,
                    op1=add,
                    accum_out=stats_t[:, pi * NB + b : pi * NB + b + 1],
                )
                w_prev = wn

    nc.sync.dma_start(out=stats_d[:], in_=stats_t[:])


def build_nc(repeat=1):
    nc = bacc.Bacc(
        "TRN2", target_bir_lowering=False, debug=False, num_devices=N_CORES
    )
    pred = nc.dram_tensor("pred", [C, HW], F32, kind="ExternalInput").ap()
    target = nc.dram_tensor("target", [C, HW], F32, kind="ExternalInput").ap()
    stats = nc.dram_tensor("stats", [P, NPAIR * NB], F32, kind="ExternalOutput").ap()
    with tile.TileContext(nc) as tc:
        with ExitStack() as ctx:
            _kernel_body(ctx, tc, stats, pred, target, repeat=repeat)
    nc.compile()
    return nc


_NC_CACHE = {}


def _get_nc():
    if "nc" not in _NC_CACHE:
        _NC_CACHE["nc"] = build_nc()
    return _NC_CACHE["nc"]


def stats_to_hists(stats):
    """[128, 48] per-core partials -> hist [2, C, NB] (pred, target) f64."""
    lo = stats[:HP].astype(np.float64).sum(axis=0).reshape(NPAIR, NB)
    hi = stats[HP:].astype(np.float64).sum(axis=0).reshape(NPAIR, NB)
    scales = np.array([_bin_scale(k) for k in range(NB)])
    hist = np.empty((2, C, NB), np.float64)
    hist[1] = lo / scales   # targets on lower partitions
    hist[0] = hi / scales   # preds on upper partitions
    return hist


def finish_on_host(stats_list):
    diffs = []
    for stats in stats_list:
        hist = stats_to_hists(stats)
        hist_n = hist / (hist.sum(axis=-1, keepdims=True) + 1e-7)
        diffs.append(np.abs(hist_n[0] - hist_n[1]))
    return np.array(np.mean(np.stack(diffs)), dtype=np.float32)


def run(pred, target, **spmd_kwargs):
    nc = _get_nc()
    pred = np.ascontiguousarray(np.asarray(pred, dtype=np.float32))
    target = np.ascontiguousarray(np.asarray(target, dtype=np.float32))
    assert pred.shape == (N_CORES, C, H, W), pred.shape
    in_maps = [
        {
            "pred": pred[b].reshape(C, HW),
            "target": target[b].reshape(C, HW),
        }
        for b in range(N_CORES)
    ]
    res = run_bass_kernel_spmd(nc, in_maps, core_ids=list(range(N_CORES)), **spmd_kwargs)
    loss = finish_on_host([res.results[b]["stats"] for b in range(N_CORES)])
    return loss, res


def kernel(pred, target):
    loss, _ = run(pred, target)
    return loss
